# revision 20
# baseline (speedup 1.0000x reference)
"""Spatial self-attention (SAGAN-style) kernel for 8 Trainium2 NeuronCores.

Math (per batch b):
    xf  = x[b].reshape(C, N)                       # C=256, N=4096
    qT  = wq @ xf + bq                             # [32, N]
    kT  = wk @ xf + bk                             # [32, N]
    V   = wv @ xf + bv                             # [C, N]
    E^T = kT.T @ qT                                # [m, n]  (keys on partitions)
    A'  = exp(E^T)          (no max-subtraction: |E| < 29, safe in fp32)
    s   = colsum(A')                               # softmax denominator per query
    out = gamma * (V @ A / s) + x

Sharding: core i handles batch b = i//2, query half h = i%2 (2048 queries).
Each core computes kT / V^T for the full 4096 keys of its batch. The host
rotates xf per-core so the core's 2048 query columns always sit at columns
0..2047 (attention reductions are permutation-invariant over keys), which
keeps the SPMD program uniform with no separate query tensor.

Device layout choices:
  - E^T orientation (keys on PSUM partitions) so A' feeds the output matmul
    as the moving operand with zero transposes anywhere.
  - V^T [m, c] computed directly on PE (lhsT = xf block, rhs = (g*wv).T).
  - softmax denominator via an all-ones [128,128] stationary matmul that
    accumulates alongside the two output-channel matmuls -> s replicated
    across all 128 partitions for free.
  - gamma folded into wv/bv on the host; residual added on DVE.
  - all matmuls in bf16 with fp32 PSUM accumulation (bf16 LDWEIGHTS gets
    fast-weight-load and hides behind the matmul; fp32/fp32r LDWEIGHTS
    cannot). Weights are pre-cast to bf16 on the host; xf is cast on DVE.
  - E PSUM tiles are [128,1024] pairs so one ACT instruction exponentiates
    two banks (amortizes the per-op ACT overhead).
  - walrus allows at most ONE semaphore wait per TPB instruction; Tile's
    surplus waits are legalized post-hoc (_strip_self_waits drops redundant
    same-engine waits, _split_multi_waits moves the rest onto single-wait
    Drains inserted before the offender).
"""

import ml_dtypes
import numpy as np

import concourse.bass as bass
import concourse.mybir as mybir
import concourse.tile as tile
from concourse.bass import ts
from concourse.bass_utils import run_bass_kernel_spmd

B, C, HH, WW = 4, 256, 64, 64
N = HH * WW          # 4096 spatial positions
D = 32               # C // 8 head dim
NCORES = 8
NQ = N * B // NCORES  # 2048 queries per core
MB = N // 128        # 32 key blocks
QCH = NQ // 512      # 4 query chunks per core
KCH = N // 512       # 8 chunks across keys

F32 = mybir.dt.float32
F32R = mybir.dt.float32r
BF16 = mybir.dt.bfloat16
AF = mybir.ActivationFunctionType
OP = mybir.AluOpType


def _build():
    nc = bass.Bass()
    xf = nc.declare_dram_parameter("xf", [C, N], F32, isOutput=False)
    wqT = nc.declare_dram_parameter("wqT", [C, D], BF16, isOutput=False)
    wkT = nc.declare_dram_parameter("wkT", [C, D], BF16, isOutput=False)
    wvT = nc.declare_dram_parameter("wvT", [C, C], BF16, isOutput=False)
    bq4 = nc.declare_dram_parameter("bq4", [128, 1], F32, isOutput=False)
    bk4 = nc.declare_dram_parameter("bk4", [128, 1], F32, isOutput=False)
    bvr = nc.declare_dram_parameter("bvr", [128, C], F32, isOutput=False)
    out = nc.declare_dram_parameter("out", [C, NQ], F32, isOutput=True)

    with tile.TileContext(nc) as tc:
        with (
            tc.tile_pool(name="const", bufs=1) as constp,
            tc.tile_pool(name="xfp", bufs=1) as xfp,
            tc.tile_pool(name="big", bufs=1) as bigp,
            tc.tile_pool(name="apool", bufs=4) as apool,
            tc.tile_pool(name="fin", bufs=2) as finp,
            tc.tile_pool(name="psO", bufs=1, space="PSUM") as psO,
            tc.tile_pool(name="psE", bufs=2, space="PSUM") as psE,
        ):
            xf_s = [xfp.tile([128, N], F32, name=f"xfs{i}") for i in range(2)]
            xf_b = [xfp.tile([128, N], BF16, name=f"xfb{i}") for i in range(2)]
            wq_t = [constp.tile([128, D], BF16, name=f"wq{i}") for i in range(2)]
            wk_t = [constp.tile([128, D], BF16, name=f"wk{i}") for i in range(2)]
            wv_t = [constp.tile([128, C], BF16, name=f"wv{i}") for i in range(2)]
            bq_t = constp.tile([128, 1], F32, name="bq")
            bk_t = constp.tile([128, 1], F32, name="bk")
            bv_t = constp.tile([128, C], F32, name="bv")
            ones_t = constp.tile([128, 128], BF16, name="ones")
            # kT/qT: rows 0:32 computed, replicated to rows 32:128 for the
            # 4x row-packed E matmuls; vT block m at cols [m*C, (m+1)*C)
            kT = bigp.tile([128, N], BF16, name="kT")
            qT = bigp.tile([128, NQ], BF16, name="qT")
            vT = bigp.tile([128, MB * C], BF16, name="vT")

            for i in range(2):
                nc.sync.dma_start(wq_t[i][:], wqT[i * 128:(i + 1) * 128, :])
                nc.sync.dma_start(wk_t[i][:], wkT[i * 128:(i + 1) * 128, :])
                nc.sync.dma_start(wv_t[i][:], wvT[i * 128:(i + 1) * 128, :])
            nc.sync.dma_start(bq_t[:], bq4[:, :])
            nc.sync.dma_start(bk_t[:], bk4[:, :])
            nc.sync.dma_start(bv_t[:], bvr[:, :])
            nc.vector.memset(ones_t[:], 1.0)
            # xf: fp32 for the residual, bf16 cast for the matmuls; split in
            # column halves so early matmuls unblock before the whole map lands
            HN = N // 2
            for i in range(2):
                for hc in range(2):
                    cs = slice(hc * HN, (hc + 1) * HN)
                    nc.sync.dma_start(xf_s[i][:, cs], xf[i * 128:(i + 1) * 128, cs])
                    nc.vector.tensor_copy(xf_b[i][:, cs], xf_s[i][:, cs])

            # phase 1: qT, kT, vT projections (PSUM extracted on DVE)
            for ch in range(QCH):
                ps = psE.tile([D, 512], F32, tag="e", name=f"psq{ch}")
                for cb in range(2):
                    nc.tensor.matmul(
                        ps[:],
                        lhsT=wq_t[cb][:],
                        rhs=xf_b[cb][:, ts(ch, 512)],
                        start=(cb == 0), stop=(cb == 1),
                    )
                nc.vector.tensor_scalar_add(qT[0:D, ts(ch, 512)], ps[:],
                                            bq_t[0:D, 0:1])
            for ch in range(KCH):
                ps = psE.tile([D, 512], F32, tag="e", name=f"psk{ch}")
                for cb in range(2):
                    nc.tensor.matmul(
                        ps[:],
                        lhsT=wk_t[cb][:],
                        rhs=xf_b[cb][:, ts(ch, 512)],
                        start=(cb == 0), stop=(cb == 1),
                    )
                nc.vector.tensor_scalar_add(kT[0:D, ts(ch, 512)], ps[:],
                                            bk_t[0:D, 0:1])
            # replicate rows 0:32 into the other three row groups
            for j in range(1, 4):
                nc.sync.dma_start(qT[32 * j:32 * (j + 1), :], qT[0:D, :])
                nc.sync.dma_start(kT[32 * j:32 * (j + 1), :], kT[0:D, :])
            for m in range(MB):
                psv = psE.tile([128, C], F32, tag="e", name=f"psv{m}")
                for cb in range(2):
                    nc.tensor.matmul(
                        psv[:],
                        lhsT=xf_b[cb][:, ts(m, 128)],
                        rhs=wv_t[cb][:],
                        start=(cb == 0), stop=(cb == 1),
                    )
                nc.vector.tensor_tensor(vT[:, ts(m, C)], psv[:], bv_t[:], OP.add)

            # phase 2: E^T -> exp -> V@A + colsum, one 512-query chunk at a time
            for ch in range(QCH):
                oc = [psO.tile([128, 512], F32, tag=f"oc{j}", name=f"oc{j}_{ch}")
                      for j in range(3)]
                for g in range(MB // 4):
                    ats = []
                    for p in range(2):          # two [128,1024] PSUM pairs
                        e = psE.tile([128, 1024], F32, tag="e",
                                     name=f"e{ch}_{g}_{p}")
                        for i in range(2):
                            mi = 4 * g + 2 * p + i
                            nc.tensor.matmul(
                                e[:, 512 * i:512 * (i + 1)],
                                lhsT=kT[32 * (2 * p + i):32 * (2 * p + i + 1),
                                        ts(mi, 128)],
                                rhs=qT[32 * (2 * p + i):32 * (2 * p + i + 1),
                                       ts(ch, 512)],
                                start=True, stop=True, skip_group_check=True,
                                tile_position=(32 * (2 * p + i), 0),
                            )
                        a = apool.tile([128, 1024], BF16, tag="a",
                                       name=f"a{ch}_{g}_{p}")
                        nc.scalar.activation(a[:], e[:], AF.Exp)
                        ats.append(a)
                    for i in range(4):
                        m = 4 * g + i
                        st, sp = (m == 0), (m == MB - 1)
                        a_r = ats[i // 2][:, 512 * (i % 2):512 * (i % 2 + 1)]
                        nc.tensor.matmul(
                            oc[0][:], lhsT=vT[:, m * C:m * C + 128],
                            rhs=a_r, start=st, stop=sp, skip_group_check=True)
                        nc.tensor.matmul(
                            oc[1][:], lhsT=vT[:, m * C + 128:(m + 1) * C],
                            rhs=a_r, start=st, stop=sp, skip_group_check=True)
                        nc.tensor.matmul(
                            oc[2][:], lhsT=ones_t[:],
                            rhs=a_r, start=st, stop=sp, skip_group_check=True)
                r = finp.tile([128, 512], F32, tag="r", name=f"r{ch}")
                nc.vector.reciprocal(r[:], oc[2][:])
                for cb in range(2):
                    t = finp.tile([128, 512], F32, tag="t", name=f"t{ch}_{cb}")
                    nc.vector.tensor_tensor(t[:], oc[cb][:], r[:], OP.mult)
                    f = finp.tile([128, 512], F32, tag="f", bufs=4,
                                  name=f"f{ch}_{cb}")
                    nc.vector.tensor_tensor(f[:], t[:],
                                            xf_s[cb][:, ts(ch, 512)],
                                            OP.add)
                    nc.gpsimd.dma_start(out[cb * 128:(cb + 1) * 128, ts(ch, 512)],
                                        f[:])
    _strip_self_waits(nc)
    _split_multi_waits(nc)
    return nc


_ENGINE_SEM_PREFIX = {
    "EngineType.PE": "PE_",
    "EngineType.DVE": "DVE_",
    "EngineType.Activation": "Activation_",
    "EngineType.Pool": "Pool_",
    "EngineType.SP": "SP_",
}


def _strip_self_waits(nc):
    """Drop same-engine semaphore waits from multi-wait TPB instructions.

    Walrus allows exactly one sync wait per TPB instruction. Tile emits
    redundant self-engine waits (WAW on pool-slot reuse, RAW from same-engine
    producers): each engine executes its queue in order, so a wait on the
    engine's own semaphore is always satisfied by program order. Dropping
    them collapses every instruction to at most one (cross-engine) wait.
    """
    for bb in nc.m.functions[0].blocks:
        for inst in bb.instructions:
            si = inst.sync_info
            if si is None:
                continue
            w = si.on_wait
            if len(w) <= 1 or inst.opcode == "Drain":
                continue
            pfx = _ENGINE_SEM_PREFIX.get(str(inst.engine))
            if pfx is None:
                continue
            kept = [x for x in w if not x.ant_name.startswith(pfx)]
            if kept and len(kept) < len(w):
                si.on_wait = kept


def _split_multi_waits(nc):
    """Walrus allows one sync wait per TPB instruction; move surplus waits
    onto dedicated single-wait Drain instructions inserted just before the
    offender (same engine, executes in order)."""
    import bass_rust
    cnt = 0
    for bb in nc.m.functions[0].blocks:
        il = bb.instructions
        i = 0
        while i < len(il):
            inst = il[i]
            si = inst.sync_info
            w = si.on_wait if si else []
            if len(w) > 1:
                for j, wait in enumerate(w[:-1]):
                    d = mybir.InstDrain(name=f"{inst.name}-w{j}", ins=[], outs=[],
                                        bass_is_fusable=False)
                    d.engine = inst.engine
                    d.sync_info = bass_rust.SyncInfo(on_wait=[wait], on_update=[])
                    il.insert(i, d)
                    i += 1
                    cnt += 1
                si.on_wait = [w[-1]]
            i += 1
    return cnt


def audit_matmul_waits(nc):
    """Max sync-wait count on any Matmult (walrus limit: 1)."""
    worst = (0, None)
    for bb in nc.m.functions[0].blocks:
        for inst in bb.instructions:
            if inst.opcode != "Matmult":
                continue
            w = inst.sync_info.on_wait if inst.sync_info else []
            if len(w) > worst[0]:
                worst = (len(w), (inst.name, [x.ant_name for x in w]))
    return worst


_NC_CACHE = None


def _get_nc():
    global _NC_CACHE
    if _NC_CACHE is None:
        _NC_CACHE = _build()
    return _NC_CACHE


def kernel(x, wq, bq, wk, bk, wv, bv, gamma, _trace=False):
    f32 = lambda a: np.ascontiguousarray(np.asarray(a, dtype=np.float32))
    x = f32(x)
    g = float(np.asarray(gamma).reshape(-1)[0])
    xfull = x.reshape(B, C, N)
    bf16 = lambda a: np.ascontiguousarray(np.asarray(a, dtype=np.float32)
                                          .astype(ml_dtypes.bfloat16))
    shared = {
        "wqT": bf16(np.asarray(wq).T),
        "wkT": bf16(np.asarray(wk).T),
        "wvT": bf16((g * np.asarray(wv)).T),
        "bq4": f32(np.tile(np.asarray(bq).reshape(D, 1), (128 // D, 1))),
        "bk4": f32(np.tile(np.asarray(bk).reshape(D, 1), (128 // D, 1))),
        "bvr": f32(np.tile((g * np.asarray(bv)).reshape(1, C), (128, 1))),
    }
    in_maps = []
    for core in range(NCORES):
        b, h = core // 2, core % 2
        m = dict(shared)
        if h == 0:
            m["xf"] = f32(xfull[b])
        else:
            # rotate so this core's query half sits at columns 0..NQ-1;
            # key order is irrelevant (attention reduces over all keys)
            m["xf"] = f32(np.concatenate(
                [xfull[b][:, NQ:], xfull[b][:, :NQ]], axis=1))
        in_maps.append(m)

    res = run_bass_kernel_spmd(_get_nc(), in_maps, list(range(NCORES)),
                               trace=_trace)
    full = np.empty((B, C, N), np.float32)
    for core in range(NCORES):
        b, h = core // 2, core % 2
        full[b][:, h * NQ:(h + 1) * NQ] = res.results[core]["out"]
    out = full.reshape(B, C, HH, WW)
    if _trace:
        return out, res
    return out


# revision 21
# speedup vs baseline: 1.4236x; 1.4236x over previous
"""Spatial self-attention (SAGAN-style) kernel for 8 Trainium2 NeuronCores.

Math (per batch b):
    xf  = x[b].reshape(C, N)                       # C=256, N=4096
    qT  = wq @ xf + bq                             # [32, N]
    kT  = wk @ xf + bk                             # [32, N]
    V   = wv @ xf + bv                             # [C, N]
    E^T = kT.T @ qT                                # [m, n]  (keys on partitions)
    A'  = exp(E^T)          (no max-subtraction: |E| < 29, safe in fp32)
    s   = colsum(A')                               # softmax denominator per query
    out = gamma * (V @ A / s) + x

Sharding: core i handles batch b = i//2, query half h = i%2 (2048 queries).
Each core computes kT / V^T for the full 4096 keys of its batch. The host
rotates xf per-core so the core's 2048 query columns always sit at columns
0..2047 (attention reductions are permutation-invariant over keys), which
keeps the SPMD program uniform with no separate query tensor.

Device layout choices:
  - E^T orientation (keys on PSUM partitions) so A' feeds the output matmul
    as the moving operand with zero transposes anywhere.
  - V^T [m, c] computed directly on PE (lhsT = xf block, rhs = (g*wv).T).
  - softmax denominator via an all-ones [128,128] stationary matmul that
    accumulates alongside the two output-channel matmuls -> s replicated
    across all 128 partitions for free.
  - gamma folded into wv/bv on the host; residual added on DVE.
  - all matmuls in bf16 with fp32 PSUM accumulation (bf16 LDWEIGHTS gets
    fast-weight-load and hides behind the matmul; fp32/fp32r LDWEIGHTS
    cannot). Weights are pre-cast to bf16 on the host; xf is cast on DVE.
  - E PSUM tiles are [128,1024] pairs so one ACT instruction exponentiates
    two banks (amortizes the per-op ACT overhead).
  - walrus allows at most ONE semaphore wait per TPB instruction; Tile's
    surplus waits are legalized post-hoc (_strip_self_waits drops redundant
    same-engine waits, _split_multi_waits moves the rest onto single-wait
    Drains inserted before the offender).
"""

import ml_dtypes
import numpy as np

import concourse.bass as bass
import concourse.mybir as mybir
import concourse.tile as tile
from concourse.bass import ts
from concourse.bass_utils import run_bass_kernel_spmd

B, C, HH, WW = 4, 256, 64, 64
N = HH * WW          # 4096 spatial positions
D = 32               # C // 8 head dim
NCORES = 8
NQ = N * B // NCORES  # 2048 queries per core
MB = N // 128        # 32 key blocks
QCH = NQ // 512      # 4 query chunks per core
KCH = N // 512       # 8 chunks across keys

F32 = mybir.dt.float32
F32R = mybir.dt.float32r
BF16 = mybir.dt.bfloat16
AF = mybir.ActivationFunctionType
OP = mybir.AluOpType


def _build():
    nc = bass.Bass()
    xf = nc.declare_dram_parameter("xf", [C, N], F32, isOutput=False)
    wqT = nc.declare_dram_parameter("wqT", [C, D], BF16, isOutput=False)
    wkT = nc.declare_dram_parameter("wkT", [C, D], BF16, isOutput=False)
    wvT = nc.declare_dram_parameter("wvT", [C, C], BF16, isOutput=False)
    bq4 = nc.declare_dram_parameter("bq4", [128, 1], F32, isOutput=False)
    bk4 = nc.declare_dram_parameter("bk4", [128, 1], F32, isOutput=False)
    bvr = nc.declare_dram_parameter("bvr", [128, C], F32, isOutput=False)
    out = nc.declare_dram_parameter("out", [C, NQ], F32, isOutput=True)

    with tile.TileContext(nc) as tc:
        with (
            tc.tile_pool(name="const", bufs=1) as constp,
            tc.tile_pool(name="xfp", bufs=1) as xfp,
            tc.tile_pool(name="big", bufs=1) as bigp,
            tc.tile_pool(name="apool", bufs=6) as apool,
            tc.tile_pool(name="fin", bufs=2) as finp,
            tc.tile_pool(name="psO", bufs=1, space="PSUM") as psO,
            tc.tile_pool(name="psE", bufs=5, space="PSUM") as psE,
        ):
            xf_s = [xfp.tile([128, N], F32, name=f"xfs{i}") for i in range(2)]
            xf_b = [xfp.tile([128, N], BF16, name=f"xfb{i}") for i in range(2)]
            wq_t = [constp.tile([128, D], BF16, name=f"wq{i}") for i in range(2)]
            wk_t = [constp.tile([128, D], BF16, name=f"wk{i}") for i in range(2)]
            wv_t = [constp.tile([128, C], BF16, name=f"wv{i}") for i in range(2)]
            bq_t = constp.tile([128, 1], F32, name="bq")
            bk_t = constp.tile([128, 1], F32, name="bk")
            bv_t = constp.tile([128, C], F32, name="bv")
            ones_t = constp.tile([128, 128], BF16, name="ones")
            # kT/qT: rows 0:32 computed, replicated to rows 32:128 for the
            # 4x row-packed E matmuls; vT block m at cols [m*C, (m+1)*C)
            kT = bigp.tile([128, N], BF16, name="kT")
            qT = bigp.tile([128, NQ], BF16, name="qT")
            vT = bigp.tile([128, MB * C], BF16, name="vT")

            for i in range(2):
                nc.sync.dma_start(wq_t[i][:], wqT[i * 128:(i + 1) * 128, :])
                nc.sync.dma_start(wk_t[i][:], wkT[i * 128:(i + 1) * 128, :])
                nc.sync.dma_start(wv_t[i][:], wvT[i * 128:(i + 1) * 128, :])
            nc.sync.dma_start(bq_t[:], bq4[:, :])
            nc.sync.dma_start(bk_t[:], bk4[:, :])
            nc.sync.dma_start(bv_t[:], bvr[:, :])
            nc.vector.memset(ones_t[:], 1.0)
            # xf: fp32 for the residual, bf16 cast for the matmuls; split in
            # column halves so early matmuls unblock before the whole map lands
            HN = N // 2
            for i in range(2):
                for hc in range(2):
                    cs = slice(hc * HN, (hc + 1) * HN)
                    nc.sync.dma_start(xf_s[i][:, cs], xf[i * 128:(i + 1) * 128, cs])
                    nc.vector.tensor_copy(xf_b[i][:, cs], xf_s[i][:, cs])

            # phase 1: qT, kT, vT projections (PSUM extracted on DVE)
            for ch in range(QCH):           # q and k interleaved per chunk
                psq = psE.tile([D, 512], F32, tag="e", name=f"psq{ch}")
                psk = psE.tile([D, 512], F32, tag="e", name=f"psk{ch}")
                for cb in range(2):
                    nc.tensor.matmul(
                        psq[:], lhsT=wq_t[cb][:], rhs=xf_b[cb][:, ts(ch, 512)],
                        start=(cb == 0), stop=(cb == 1), skip_group_check=True)
                    nc.tensor.matmul(
                        psk[:], lhsT=wk_t[cb][:], rhs=xf_b[cb][:, ts(ch, 512)],
                        start=(cb == 0), stop=(cb == 1), skip_group_check=True)
                nc.vector.tensor_scalar_add(qT[0:D, ts(ch, 512)], psq[:],
                                            bq_t[0:D, 0:1])
                nc.vector.tensor_scalar_add(kT[0:D, ts(ch, 512)], psk[:],
                                            bk_t[0:D, 0:1])
            for ch in range(QCH, KCH):
                ps = psE.tile([D, 512], F32, tag="e", name=f"psk{ch}")
                ps2 = psE.tile([D, 512], F32, tag="e", name=f"psk2_{ch}")
                h = 256
                for cb in range(2):
                    nc.tensor.matmul(
                        ps[:, 0:h], lhsT=wk_t[cb][:],
                        rhs=xf_b[cb][:, ch * 512:ch * 512 + h],
                        start=(cb == 0), stop=(cb == 1), skip_group_check=True)
                    nc.tensor.matmul(
                        ps2[:, 0:h], lhsT=wk_t[cb][:],
                        rhs=xf_b[cb][:, ch * 512 + h:(ch + 1) * 512],
                        start=(cb == 0), stop=(cb == 1), skip_group_check=True)
                nc.vector.tensor_scalar_add(kT[0:D, ch * 512:ch * 512 + h],
                                            ps[:, 0:h], bk_t[0:D, 0:1])
                nc.vector.tensor_scalar_add(kT[0:D, ch * 512 + h:(ch + 1) * 512],
                                            ps2[:, 0:h], bk_t[0:D, 0:1])
            # replicate rows 0:32 into the other three row groups
            for j in range(1, 4):
                nc.sync.dma_start(qT[32 * j:32 * (j + 1), :], qT[0:D, :])
                nc.sync.dma_start(kT[32 * j:32 * (j + 1), :], kT[0:D, :])
            for mp in range(MB // 2):       # m-pairs: alternate PSUM banks
                m0, m1 = 2 * mp, 2 * mp + 1
                psv0 = psE.tile([128, C], F32, tag="e", name=f"psv{m0}")
                psv1 = psE.tile([128, C], F32, tag="e", name=f"psv{m1}")
                for cb in range(2):
                    nc.tensor.matmul(
                        psv0[:], lhsT=xf_b[cb][:, ts(m0, 128)], rhs=wv_t[cb][:],
                        start=(cb == 0), stop=(cb == 1), skip_group_check=True)
                    nc.tensor.matmul(
                        psv1[:], lhsT=xf_b[cb][:, ts(m1, 128)], rhs=wv_t[cb][:],
                        start=(cb == 0), stop=(cb == 1), skip_group_check=True)
                nc.vector.tensor_tensor(vT[:, ts(m0, C)], psv0[:], bv_t[:], OP.add)
                nc.vector.tensor_tensor(vT[:, ts(m1, C)], psv1[:], bv_t[:], OP.add)

            # phase 2: E^T -> exp -> V@A + colsum, one 512-query chunk at a time
            for ch in range(QCH):
                oc = [psO.tile([128, 512], F32, tag=f"oc{j}", name=f"oc{j}_{ch}")
                      for j in range(3)]
                for g in range(MB // 4):
                    ats = []
                    for i in range(4):
                        m = 4 * g + i
                        e = psE.tile([128, 512], F32, tag="e", name=f"e{ch}_{m}")
                        nc.tensor.matmul(
                            e[:],
                            lhsT=kT[32 * i:32 * (i + 1), ts(m, 128)],
                            rhs=qT[32 * i:32 * (i + 1), ts(ch, 512)],
                            start=True, stop=True, skip_group_check=True,
                            tile_position=(32 * i, 0),
                        )
                        a = apool.tile([128, 512], BF16, tag="a",
                                       name=f"a{ch}_{m}")
                        nc.scalar.activation(a[:], e[:], AF.Exp)
                        ats.append(a)
                    for i in range(4):
                        m = 4 * g + i
                        st, sp = (m == 0), (m == MB - 1)
                        nc.tensor.matmul(
                            oc[0][:], lhsT=vT[:, m * C:m * C + 128],
                            rhs=ats[i][:], start=st, stop=sp, skip_group_check=True)
                        nc.tensor.matmul(
                            oc[1][:], lhsT=vT[:, m * C + 128:(m + 1) * C],
                            rhs=ats[i][:], start=st, stop=sp, skip_group_check=True)
                        nc.tensor.matmul(
                            oc[2][:], lhsT=ones_t[:],
                            rhs=ats[i][:], start=st, stop=sp, skip_group_check=True)
                r = finp.tile([128, 512], F32, tag="r", name=f"r{ch}")
                nc.vector.reciprocal(r[:], oc[2][:])
                for cb in range(2):
                    t = finp.tile([128, 512], F32, tag="t", name=f"t{ch}_{cb}")
                    nc.vector.tensor_tensor(t[:], oc[cb][:], r[:], OP.mult)
                    f = finp.tile([128, 512], F32, tag="f", bufs=4,
                                  name=f"f{ch}_{cb}")
                    nc.vector.tensor_tensor(f[:], t[:],
                                            xf_s[cb][:, ts(ch, 512)],
                                            OP.add)
                    nc.gpsimd.dma_start(out[cb * 128:(cb + 1) * 128, ts(ch, 512)],
                                        f[:])
    _strip_self_waits(nc)
    _split_multi_waits(nc)
    return nc


_ENGINE_SEM_PREFIX = {
    "EngineType.PE": "PE_",
    "EngineType.DVE": "DVE_",
    "EngineType.Activation": "Activation_",
    "EngineType.Pool": "Pool_",
    "EngineType.SP": "SP_",
}


def _strip_self_waits(nc):
    """Drop same-engine semaphore waits from multi-wait TPB instructions.

    Walrus allows exactly one sync wait per TPB instruction. Tile emits
    redundant self-engine waits (WAW on pool-slot reuse, RAW from same-engine
    producers): each engine executes its queue in order, so a wait on the
    engine's own semaphore is always satisfied by program order. Dropping
    them collapses every instruction to at most one (cross-engine) wait.
    """
    for bb in nc.m.functions[0].blocks:
        for inst in bb.instructions:
            si = inst.sync_info
            if si is None:
                continue
            w = si.on_wait
            if len(w) <= 1 or inst.opcode == "Drain":
                continue
            pfx = _ENGINE_SEM_PREFIX.get(str(inst.engine))
            if pfx is None:
                continue
            kept = [x for x in w if not x.ant_name.startswith(pfx)]
            if kept and len(kept) < len(w):
                si.on_wait = kept


def _split_multi_waits(nc):
    """Walrus allows one sync wait per TPB instruction; move surplus waits
    onto dedicated single-wait Drain instructions inserted just before the
    offender (same engine, executes in order)."""
    import bass_rust
    cnt = 0
    for bb in nc.m.functions[0].blocks:
        il = bb.instructions
        i = 0
        while i < len(il):
            inst = il[i]
            si = inst.sync_info
            w = si.on_wait if si else []
            if len(w) > 1:
                for j, wait in enumerate(w[:-1]):
                    d = mybir.InstDrain(name=f"{inst.name}-w{j}", ins=[], outs=[],
                                        bass_is_fusable=False)
                    d.engine = inst.engine
                    d.sync_info = bass_rust.SyncInfo(on_wait=[wait], on_update=[])
                    il.insert(i, d)
                    i += 1
                    cnt += 1
                si.on_wait = [w[-1]]
            i += 1
    return cnt


def audit_matmul_waits(nc):
    """Max sync-wait count on any Matmult (walrus limit: 1)."""
    worst = (0, None)
    for bb in nc.m.functions[0].blocks:
        for inst in bb.instructions:
            if inst.opcode != "Matmult":
                continue
            w = inst.sync_info.on_wait if inst.sync_info else []
            if len(w) > worst[0]:
                worst = (len(w), (inst.name, [x.ant_name for x in w]))
    return worst


_NC_CACHE = None


def _get_nc():
    global _NC_CACHE
    if _NC_CACHE is None:
        _NC_CACHE = _build()
    return _NC_CACHE


def kernel(x, wq, bq, wk, bk, wv, bv, gamma, _trace=False):
    f32 = lambda a: np.ascontiguousarray(np.asarray(a, dtype=np.float32))
    x = f32(x)
    g = float(np.asarray(gamma).reshape(-1)[0])
    xfull = x.reshape(B, C, N)
    bf16 = lambda a: np.ascontiguousarray(np.asarray(a, dtype=np.float32)
                                          .astype(ml_dtypes.bfloat16))
    shared = {
        "wqT": bf16(np.asarray(wq).T),
        "wkT": bf16(np.asarray(wk).T),
        "wvT": bf16((g * np.asarray(wv)).T),
        "bq4": f32(np.tile(np.asarray(bq).reshape(D, 1), (128 // D, 1))),
        "bk4": f32(np.tile(np.asarray(bk).reshape(D, 1), (128 // D, 1))),
        "bvr": f32(np.tile((g * np.asarray(bv)).reshape(1, C), (128, 1))),
    }
    in_maps = []
    for core in range(NCORES):
        b, h = core // 2, core % 2
        m = dict(shared)
        if h == 0:
            m["xf"] = f32(xfull[b])
        else:
            # rotate so this core's query half sits at columns 0..NQ-1;
            # key order is irrelevant (attention reduces over all keys)
            m["xf"] = f32(np.concatenate(
                [xfull[b][:, NQ:], xfull[b][:, :NQ]], axis=1))
        in_maps.append(m)

    res = run_bass_kernel_spmd(_get_nc(), in_maps, list(range(NCORES)),
                               trace=_trace)
    full = np.empty((B, C, N), np.float32)
    for core in range(NCORES):
        b, h = core // 2, core % 2
        full[b][:, h * NQ:(h + 1) * NQ] = res.results[core]["out"]
    out = full.reshape(B, C, HH, WW)
    if _trace:
        return out, res
    return out
